# revision 1
# baseline (speedup 1.0000x reference)
"""DSGIAT GraphBranch kernel for trn2 (8 NeuronCores).

Device: channel-sharded conv1 GEMM (x @ W1 shard + folded attention-logit
columns) across 8 cores via Bass/Tile. Host: edge aggregation via sorted
segment reduceat, LP diffusion, pooling, MLP.
"""
import numpy as np
from contextlib import ExitStack

N_NODES = 30000
N_PAD = 30080          # 235 * 128
N_TILES = 235
IN_CH = 256
HID = 128
HEADS = 4
OUT1 = 512
N_GRAPHS = 64
LP_ALPHA = 0.5
NEG_SLOPE = 0.2
EPS = 1e-16
N_CORES = 8
SHARD = OUT1 // N_CORES  # 64

_cached = {}


def _build_device_program():
    import concourse.tile as tile
    from concourse import bacc, mybir

    nc = bacc.Bacc("TRN2", target_bir_lowering=False, debug=False,
                   num_devices=N_CORES)
    xT = nc.dram_tensor("xT", [IN_CH, N_PAD], mybir.dt.float32,
                        kind="ExternalInput")
    w1rhs = nc.dram_tensor("w1rhs", [IN_CH, 66], mybir.dt.float32,
                           kind="ExternalInput")
    out_h = nc.dram_tensor("out_h", [N_PAD, 66], mybir.dt.float32,
                           kind="ExternalOutput")

    with tile.TileContext(nc) as tc, ExitStack() as ctx:
        sb = ctx.enter_context(tc.tile_pool(name="sb", bufs=4))
        wp = ctx.enter_context(tc.tile_pool(name="wp", bufs=1))
        ps = ctx.enter_context(tc.tile_pool(name="ps", bufs=4, space="PSUM"))

        w_sb = wp.tile([128, 2, 66], mybir.dt.float32)
        nc.sync.dma_start(w_sb[:, 0, :], w1rhs[0:128, :])
        nc.sync.dma_start(w_sb[:, 1, :], w1rhs[128:256, :])

        for j in range(N_TILES):
            acc = ps.tile([128, 66], mybir.dt.float32, space="PSUM")
            for q in range(2):
                lhsT = sb.tile([128, 128], mybir.dt.float32, tag="lhsT")
                nc.sync.dma_start(
                    lhsT[:], xT[q * 128:(q + 1) * 128, j * 128:(j + 1) * 128])
                nc.tensor.matmul(acc[:], lhsT=lhsT[:], rhs=w_sb[:, q, :],
                                 start=(q == 0), stop=(q == 1))
            res = sb.tile([128, 66], mybir.dt.float32, tag="res")
            nc.vector.tensor_copy(res[:], acc[:])
            nc.sync.dma_start(out_h[j * 128:(j + 1) * 128, :], res[:])
    nc.compile()
    return nc


def _run_device(xT, w1rhs_list):
    from concourse.bass_utils import run_bass_kernel_spmd
    if "nc" not in _cached:
        _cached["nc"] = _build_device_program()
    nc = _cached["nc"]
    in_maps = [{"xT": xT, "w1rhs": w1rhs_list[c]} for c in range(N_CORES)]
    import time
    t0 = time.time()
    res = run_bass_kernel_spmd(nc, in_maps, core_ids=list(range(N_CORES)))
    _cached["device_wall_ns"] = int((time.time() - t0) * 1e9)
    _cached["last_result"] = res
    return [res.results[c]["out_h"] for c in range(N_CORES)]


def _seg_sum(vals, starts, n_seg):
    """Segment sum of vals over sorted segments; starts has n_seg entries."""
    out = np.add.reduceat(vals, starts, axis=0)
    # fix empty segments (reduceat returns vals[start] there)
    counts = np.diff(np.append(starts, len(vals)))
    if vals.ndim == 1:
        out = np.where(counts > 0, out, 0.0)
    else:
        out = np.where((counts > 0)[:, None], out, 0.0)
    return out


def _seg_max(vals, starts, n_seg):
    out = np.maximum.reduceat(vals, starts, axis=0)
    counts = np.diff(np.append(starts, len(vals)))
    out = np.where((counts > 0)[:, None], out, 0.0)
    return out


def _sorted_edges(src, dst, n):
    order = np.argsort(dst, kind="stable")
    s, d = src[order], dst[order]
    starts = np.searchsorted(d, np.arange(n))
    return s, d, starts


def _gat_agg(h, es, ed, src, dst, n):
    """h [N,512], es/ed [N,4]; edges include self loops, any order."""
    s, d, starts = _sorted_edges(src, dst, n)
    e = es[s] + ed[d]                                 # [E,4]
    e = np.where(e >= 0, e, NEG_SLOPE * e)
    m = _seg_max(e, starts, n)                        # [N,4]
    a = np.exp(e - m[d])                              # [E,4]
    denom = _seg_sum(a, starts, n)                    # [N,4]
    hh = h.reshape(n, HEADS, HID)
    msg = hh[s] * a[:, :, None]                       # [E,4,128]
    outs = _seg_sum(msg.reshape(len(s), -1), starts, n).reshape(n, HEADS, HID)
    outs = outs / (denom[:, :, None] + EPS)
    return outs.reshape(n, OUT1)


def _label_prop(y, src, dst, dis, n):
    s, d, starts = _sorted_edges(src, dst, n)
    w = (dis[s] * dis[d])[:, None]
    res = (1.0 - LP_ALPHA) * y
    out = y
    for _ in range(2):
        agg = _seg_sum(out[s] * w, starts, n)
        out = np.clip(LP_ALPHA * agg + res, 0.0, 1.0)
    return out


def kernel(x, edge_index, batch,
           conv1_W, conv1_asrc, conv1_adst, conv1_b,
           conv2_W, conv2_asrc, conv2_adst, conv2_b,
           mlp_W1, mlp_b1, mlp_W2, mlp_b2):
    x = np.asarray(x, dtype=np.float32)
    edge_index = np.asarray(edge_index)
    batch = np.asarray(batch)
    conv1_W = np.asarray(conv1_W, np.float32)
    conv2_W = np.asarray(conv2_W, np.float32)
    n = x.shape[0]
    src = edge_index[0].astype(np.int64)
    dst = edge_index[1].astype(np.int64)
    loop = np.arange(n, dtype=np.int64)
    c_src = np.concatenate([src, loop])
    c_dst = np.concatenate([dst, loop])

    # degrees / LP normalization
    deg = np.bincount(dst, minlength=n).astype(np.float32)
    dis = np.where(deg > 0, 1.0 / np.sqrt(np.maximum(deg, 1.0)), 0.0)

    # ---- device phase: conv1 GEMM, channel-sharded across 8 cores ----
    # folded logit weights: e_s = x @ (W1_head @ a_src)
    a1s = np.asarray(conv1_asrc, np.float32)  # [4,128]
    a1d = np.asarray(conv1_adst, np.float32)
    w_es1 = np.stack([conv1_W[:, h * HID:(h + 1) * HID] @ a1s[h]
                      for h in range(HEADS)], axis=1)  # [256,4]
    w_ed1 = np.stack([conv1_W[:, h * HID:(h + 1) * HID] @ a1d[h]
                      for h in range(HEADS)], axis=1)

    xp = np.zeros((N_PAD, IN_CH), dtype=np.float32)
    xp[:n] = x
    xT = np.ascontiguousarray(xp.T)  # [256, 30080]
    w1rhs_list = []
    for c in range(N_CORES):
        w = np.zeros((IN_CH, 66), dtype=np.float32)
        w[:, 0:SHARD] = conv1_W[:, c * SHARD:(c + 1) * SHARD]
        h_head = c // 2
        w[:, 64] = w_es1[:, h_head]
        w[:, 65] = w_ed1[:, h_head]
        w1rhs_list.append(w)

    outs = _run_device(xT, w1rhs_list)
    h1_pre = np.concatenate([o[:n, 0:SHARD] for o in outs], axis=1)  # [N,512]
    es1 = np.stack([outs[2 * h][:n, 64] for h in range(HEADS)], axis=1)
    ed1 = np.stack([outs[2 * h][:n, 65] for h in range(HEADS)], axis=1)

    # ---- host: conv1 aggregation + relu + LP ----
    h1 = _gat_agg(h1_pre, es1, ed1, c_src, c_dst, n) + np.asarray(conv1_b, np.float32)
    h1 = np.maximum(h1, 0.0)
    h1 = _label_prop(h1, src, dst, dis, n)

    # ---- host: conv2 ----
    h2_pre = h1 @ conv2_W
    a2s = np.asarray(conv2_asrc, np.float32)
    a2d = np.asarray(conv2_adst, np.float32)
    hh = h2_pre.reshape(n, HEADS, HID)
    es2 = np.einsum("nhc,hc->nh", hh, a2s)
    ed2 = np.einsum("nhc,hc->nh", hh, a2d)
    h2 = _gat_agg(h2_pre, es2, ed2, c_src, c_dst, n) + np.asarray(conv2_b, np.float32)
    h2 = np.maximum(h2, 0.0)
    h2 = _label_prop(h2, src, dst, dis, n)

    # ---- pooling + MLP ----
    combined = np.concatenate([x, h1, h2], axis=1)  # [N,1280]
    b = batch.astype(np.int64)
    sums = np.zeros((N_GRAPHS, combined.shape[1]), dtype=np.float32)
    np.add.at(sums, b, combined)
    cnts = np.bincount(b, minlength=N_GRAPHS).astype(np.float32)
    pooled = sums / np.maximum(cnts, 1.0)[:, None]
    hdd = np.maximum(pooled @ np.asarray(mlp_W1, np.float32)
                     + np.asarray(mlp_b1, np.float32), 0.0)
    out = hdd @ np.asarray(mlp_W2, np.float32) + np.asarray(mlp_b2, np.float32)
    return out.astype(np.float32)



# revision 11
# speedup vs baseline: 48560.9979x; 48560.9979x over previous
"""DSGIAT GraphBranch kernel for trn2: full-device implementation on 8 cores.

Pipeline (all on device, single launch):
  GEMM1 (row-sharded, attention logits folded as extra columns) -> AllGather
  -> GAT agg (dma_gather of src rows + one-hot selector matmuls; softmax
     without max-subtraction) -> AG -> LP x2 (gather + selector matmul) with
     AG between -> GEMM2 -> AG -> GAT2 -> AG -> LP x2 -> transposed pooling
     via one-hot batch matmul -> AllReduce -> replicated MLP (transposed).

Sharding: nodes row-sharded 8 ways (3840 rows/core of 30720 padded); each
sparse pass processes edges whose dst is in the core's slab; exchanges via
ncfw AllGather. Payloads bf16, selectors fp8 ({0,1} exact), accum fp32.
"""
import os
import numpy as np
import ml_dtypes
from contextlib import ExitStack

BF16 = ml_dtypes.bfloat16
F8 = ml_dtypes.float8_e4m3

# ---- sizes (full problem; test_sim overrides via set_config) ----
CFG = dict(
    N=30000, NPAD=30720, IN_CH=256, OUT1=512, HID=128, HEADS=4,
    N_GRAPHS=64, NCORE=8,
)

NEG_SLOPE = 0.2
EPS = 1e-16
PAD = None  # set from cfg: pad gather index (a real, all-zero row)

_cached = {}


def set_config(**kw):
    CFG.update(kw)
    _cached.clear()


# ---------------- host preprocessing ----------------

def _wrap_idx(idx2d):
    """[T, EP] int -> dma_gather layout [128, T*(EP//16)] int16."""
    T, EP = idx2d.shape
    a = idx2d.reshape(T, EP // 16, 16).astype(np.int16)
    w16 = a.transpose(2, 0, 1)                      # [16, T, S]
    w = np.tile(w16, (8, 1, 1))                     # [128, T, S]
    return np.ascontiguousarray(w.reshape(128, -1))


def _edge_tiles(src, dst, ntiles, pad_idx, wgt=None):
    """Sort edges by dst, pad per dst-tile of 128. Returns
    (EP, src_pad [T,EP], dst_pad [T,EP], dstl [T,EP], w_pad or None)."""
    order = np.argsort(dst, kind="stable")
    s, d = src[order], dst[order]
    t = d // 128
    cnt = np.bincount(t, minlength=ntiles)
    EP = max(128, int(-(-cnt.max() // 128)) * 128)
    offs = np.zeros(ntiles, np.int64)
    offs[1:] = np.cumsum(cnt)[:-1]
    pos = np.arange(len(s)) - offs[t]
    sp = np.full((ntiles, EP), pad_idx, np.int32)
    dp = np.full((ntiles, EP), pad_idx, np.int32)
    dl = np.full((ntiles, EP), -1, np.int32)
    sp[t, pos] = s
    dp[t, pos] = d
    dl[t, pos] = d - t * 128
    wp = None
    if wgt is not None:
        wp = np.zeros((ntiles, EP), np.float32)
        wp[t, pos] = wgt[order]
    return EP, sp, dp, dl, wp


def _selector(dl):
    """One-hot [T, EP, 128] fp8 from dst-local indices (-1 -> zero row)."""
    T, EP = dl.shape
    S = np.zeros((T, EP, 128), np.uint8)
    ti, ei = np.nonzero(dl >= 0)
    S[ti, ei, dl[ti, ei]] = 1
    return S


def _sel_layout(S):
    """[T, EP, 128] -> [128, T*EP] fp8 (edge e=c*128+p of tile t at
    [p, t*EP + c*128 + n])."""
    T, EP, _ = S.shape
    CH = EP // 128
    out = S.reshape(T, CH, 128, 128).transpose(2, 0, 1, 3).reshape(128, -1)
    return out.astype(F8)


def _chunk_layout(v):
    """[T, EP] -> [128, T*(EP//128)]: value of edge c*128+p at [p, t*CH+c]."""
    T, EP = v.shape
    CH = EP // 128
    return np.ascontiguousarray(
        v.reshape(T, CH, 128).transpose(2, 0, 1).reshape(128, -1))


def _fold_logit_w(W, a_src, a_dst, heads, hid):
    ws = np.stack([W[:, h * hid:(h + 1) * hid] @ a_src[h] for h in range(heads)],
                  axis=1)
    wd = np.stack([W[:, h * hid:(h + 1) * hid] @ a_dst[h] for h in range(heads)],
                  axis=1)
    return ws, wd  # [in, heads]


# ---------------- device program ----------------

def _build(key):
    import concourse.tile as tile
    from concourse import bacc, mybir, bass

    EPG, EPL, NCORE = key
    c = CFG
    NPAD, IN_CH, OUT1, HEADS = c["NPAD"], c["IN_CH"], c["OUT1"], c["HEADS"]
    NG = c["N_GRAPHS"]
    NT = NPAD // 128
    TPC = NT // NCORE
    SLAB = TPC * 128
    KIN = IN_CH // 128          # k-chunks for GEMM1
    KH = OUT1 // 128            # k-chunks for GEMM2 / feature blocks
    CHG = EPG // 128
    CHL = EPL // 128
    GS = EPG // 16              # idx slots per tile (gather1)
    LS = EPL // 16
    W1C = OUT1 + 2 * HEADS      # 520
    STR = ((W1C * 2 + 255) // 256) * 128  # row stride elems (640) bf16
    JK = IN_CH + 2 * OUT1
    FCH = JK // 128             # 10 pooled feature chunks
    MLP1 = 256
    OC = MLP1 // 128            # 2

    bf = mybir.dt.bfloat16
    f32 = mybir.dt.float32
    f8 = mybir.dt.float8e4
    i16 = mybir.dt.int16

    nc = bacc.Bacc("TRN2", target_bir_lowering=False, debug=False,
                   num_devices=NCORE)

    # ---- inputs ----
    xts_d = nc.dram_tensor("xts", [IN_CH, SLAB], bf, kind="ExternalInput")
    xr_d = nc.dram_tensor("xr", [SLAB, IN_CH], bf, kind="ExternalInput")
    w1_d = nc.dram_tensor("w1e", [IN_CH, W1C], bf, kind="ExternalInput")
    w2_d = nc.dram_tensor("w2e", [OUT1, W1C], bf, kind="ExternalInput")
    b1_d = nc.dram_tensor("b1r", [128, OUT1], bf, kind="ExternalInput")
    b2_d = nc.dram_tensor("b2r", [128, OUT1], bf, kind="ExternalInput")
    gidx_d = nc.dram_tensor("gidx", [128, TPC * GS], i16, kind="ExternalInput")
    geidx_d = nc.dram_tensor("geidx", [128, TPC * 2 * GS], i16,
                             kind="ExternalInput")
    sgat_d = nc.dram_tensor("sgat", [128, TPC * EPG], f8, kind="ExternalInput")
    lidx_d = nc.dram_tensor("lidx", [128, TPC * LS], i16, kind="ExternalInput")
    slp_d = nc.dram_tensor("slp", [128, TPC * EPL], f8, kind="ExternalInput")
    wlp_d = nc.dram_tensor("wlp", [128, TPC * CHL], bf, kind="ExternalInput")
    spool_d = nc.dram_tensor("spool", [128, TPC * NG], f8, kind="ExternalInput")
    mw1_d = nc.dram_tensor("mw1", [128, FCH * OC * 128], bf,
                           kind="ExternalInput")
    mw2_d = nc.dram_tensor("mw2", [128, OC * 128], bf, kind="ExternalInput")
    mb1_d = nc.dram_tensor("mb1", [128, OC], f32, kind="ExternalInput")
    mb2_d = nc.dram_tensor("mb2", [128, 1], f32, kind="ExternalInput")
    rcnt_d = nc.dram_tensor("rcnt", [128, NG], f32, kind="ExternalInput")
    outT_d = nc.dram_tensor("outT", [128, NG], f32, kind="ExternalOutput")

    # ---- internal DRAM ----
    def idram(name, shape, dt, shared=False):
        return nc.dram_tensor(name, shape, dt, kind="Internal",
                              addr_space="Shared" if shared else "Local")

    sh = NCORE > 4
    h1p_own = idram("h1p_own", [SLAB, STR], bf)
    h1p = idram("h1p_full", [NPAD, STR], bf, shared=sh)
    h10_own = idram("h10_own", [SLAB, OUT1], bf)
    h10 = idram("h10_full", [NPAD, OUT1], bf, shared=sh)
    h1a_own = idram("h1a_own", [SLAB, OUT1], bf)
    h1a = idram("h1a_full", [NPAD, OUT1], bf, shared=sh)
    h1f_own = idram("h1f_own", [SLAB, OUT1], bf)
    h2p_own = idram("h2p_own", [SLAB, STR], bf)
    h2p = idram("h2p_full", [NPAD, STR], bf, shared=sh)
    h20_own = idram("h20_own", [SLAB, OUT1], bf)
    h20 = idram("h20_full", [NPAD, OUT1], bf, shared=sh)
    h2a_own = idram("h2a_own", [SLAB, OUT1], bf)
    h2a = idram("h2a_full", [NPAD, OUT1], bf, shared=sh)
    h2f_own = idram("h2f_own", [SLAB, OUT1], bf)
    ar_in = idram("ar_in", [128, FCH * NG], f32)
    ar_out = idram("ar_out", [128, FCH * NG], f32, shared=sh)

    ts = bass.ts
    RG = [list(range(NCORE))]
    AF = mybir.ActivationFunctionType
    ALU = mybir.AluOpType

    with tile.TileContext(nc) as tc, ExitStack() as ctx:
        cst = ctx.enter_context(tc.tile_pool(name="cst", bufs=1))
        big = ctx.enter_context(tc.tile_pool(name="big", bufs=1))
        sb = ctx.enter_context(tc.tile_pool(name="sb", bufs=2))
        ps = ctx.enter_context(tc.tile_pool(name="ps", bufs=2, space="PSUM"))
        pp = ctx.enter_context(tc.tile_pool(name="pp", bufs=2, space="PSUM"))

        # resident constants
        xts = big.tile([128, KIN, SLAB], bf, tag="glhs")
        for k in range(KIN):
            nc.sync.dma_start(xts[:, k, :], xts_d[ts(k, 128), :])
        w1 = cst.tile([128, KIN, W1C], bf)
        for k in range(KIN):
            nc.sync.dma_start(w1[:, k, :], w1_d[ts(k, 128), :])
        w2 = cst.tile([128, KH, W1C], bf)
        for k in range(KH):
            nc.sync.dma_start(w2[:, k, :], w2_d[ts(k, 128), :])
        b1r = cst.tile([128, OUT1], bf)
        nc.sync.dma_start(b1r[:], b1_d[:])
        b2r = cst.tile([128, OUT1], bf)
        nc.sync.dma_start(b2r[:], b2_d[:])
        gidx = cst.tile([128, TPC * GS], i16)
        nc.sync.dma_start(gidx[:], gidx_d[:])
        geidx = cst.tile([128, TPC * 2 * GS], i16)
        nc.sync.dma_start(geidx[:], geidx_d[:])
        lidx = cst.tile([128, TPC * LS], i16)
        nc.sync.dma_start(lidx[:], lidx_d[:])
        wlp = cst.tile([128, TPC * CHL], bf)
        nc.sync.dma_start(wlp[:], wlp_d[:])
        spool = cst.tile([128, TPC * NG], f8)
        nc.sync.dma_start(spool[:], spool_d[:])
        rcnt = cst.tile([128, NG], f32)
        nc.sync.dma_start(rcnt[:], rcnt_d[:])
        mb1 = cst.tile([128, OC], f32)
        nc.sync.dma_start(mb1[:], mb1_d[:])
        mb2 = cst.tile([128, 1], f32)
        nc.sync.dma_start(mb2[:], mb2_d[:])

        def gemm(wt, kch, src_get, dst):
            """dst[t rows] = lhsT_chunks^T @ wt ([128,kch,W1C])."""
            for t in range(TPC):
                pa = ps.tile([128, OUT1], f32, tag="acc")
                pb = ps.tile([128, 2 * HEADS], f32, tag="acc2")
                for k in range(kch):
                    lh = src_get(k, t)
                    nc.tensor.matmul(pa[:], lhsT=lh, rhs=wt[:, k, 0:OUT1],
                                     start=(k == 0), stop=(k == kch - 1))
                    nc.tensor.matmul(pb[:], lhsT=lh, rhs=wt[:, k, OUT1:W1C],
                                     start=(k == 0), stop=(k == kch - 1))
                ot = sb.tile([128, STR], bf, tag="geo")
                nc.vector.memset(ot[:, W1C:STR], 0.0)
                nc.vector.tensor_copy(ot[:, 0:OUT1], pa[:])
                nc.vector.tensor_copy(ot[:, OUT1:W1C], pb[:])
                nc.sync.dma_start(dst[ts(t, 128), :], ot[:])

        # ---- GEMM1 ----
        gemm(w1, KIN, lambda k, t: xts[:, k, ts(t, 128)], h1p_own)
        nc.gpsimd.collective_compute("AllGather", ALU.bypass,
                                     replica_groups=RG,
                                     ins=[h1p_own[:]], outs=[h1p[:]])

        def gat_pass(hp_full, out_own, out_bounce):
            for t in range(TPC):
                gh = sb.tile([128, CHG, OUT1], bf, tag="gbig")
                nc.gpsimd.dma_gather(
                    out_ap=gh[:], in_ap=hp_full[:, 0:OUT1],
                    idxs_ap=gidx[:, ts(t, GS)], num_idxs=EPG,
                    num_idxs_reg=EPG, elem_size=OUT1, elem_step=STR)
                ge = sb.tile([128, 2 * CHG, 128], bf, tag="ge")
                nc.gpsimd.dma_gather(
                    out_ap=ge[:], in_ap=hp_full[:, OUT1:STR],
                    idxs_ap=geidx[:, ts(t, 2 * GS)], num_idxs=2 * EPG,
                    num_idxs_reg=2 * EPG, elem_size=STR - OUT1,
                    elem_step=STR)
                st = sb.tile([128, CHG, 128], f8, tag="sel")
                nc.sync.dma_start(
                    st[:].rearrange("p a b -> p (a b)"), sgat_d[:, ts(t, EPG)])
                lg = sb.tile([128, CHG, HEADS], f32, tag="lg")
                nc.vector.tensor_add(lg[:], ge[:, 0:CHG, 0:HEADS],
                                     ge[:, CHG:2 * CHG, HEADS:2 * HEADS])
                # leaky_relu: max(0.2*x, x) in one DVE op
                nc.vector.scalar_tensor_tensor(
                    lg[:], lg[:], NEG_SLOPE, lg[:], ALU.mult, ALU.max)
                ex = sb.tile([128, CHG, HEADS], bf, tag="ex")
                nc.scalar.activation(ex[:], lg[:], AF.Exp)
                ghv = gh[:].rearrange("p a (h q) -> p a h q", h=HEADS)
                nc.vector.tensor_mul(
                    ghv, ghv,
                    ex[:].unsqueeze(3).broadcast_to(
                        [128, CHG, HEADS, OUT1 // HEADS]))
                pnum = ps.tile([128, OUT1], f32, tag="acc")
                pden = ps.tile([128, HEADS], f32, tag="acc2")
                for ch in range(CHG):
                    nc.tensor.matmul(pnum[:], lhsT=st[:, ch, :],
                                     rhs=gh[:, ch, :],
                                     start=(ch == 0), stop=(ch == CHG - 1))
                    nc.tensor.matmul(pden[:], lhsT=st[:, ch, :],
                                     rhs=ex[:, ch, :],
                                     start=(ch == 0), stop=(ch == CHG - 1))
                de = sb.tile([128, HEADS], f32, tag="de")
                nc.vector.tensor_scalar_add(de[:], pden[:], EPS)
                nc.vector.reciprocal(de[:], de[:])
                tmp = sb.tile([128, HEADS, OUT1 // HEADS], f32, tag="tf32")
                nc.vector.tensor_mul(
                    tmp[:], pnum[:].rearrange("p (h q) -> p h q", h=HEADS),
                    de[:].unsqueeze(2).broadcast_to(
                        [128, HEADS, OUT1 // HEADS]))
                ob = sb.tile([128, OUT1], bf, tag="obuf")
                nc.vector.tensor_add(
                    ob[:], tmp[:].rearrange("p h q -> p (h q)"), b1r[:])
                nc.vector.tensor_scalar_max(ob[:], ob[:], 0.0)
                nc.sync.dma_start(out_own[ts(t, 128), :], ob[:])
                if out_bounce is not None:
                    nc.sync.dma_start(out_bounce[ts(t, 128), :], ob[:])

        def lp_pass(h_full, res_own, out_own):
            for t in range(TPC):
                g = sb.tile([128, CHL, OUT1], bf, tag="gbig")
                nc.gpsimd.dma_gather(
                    out_ap=g[:], in_ap=h_full[:], idxs_ap=lidx[:, ts(t, LS)],
                    num_idxs=EPL, num_idxs_reg=EPL, elem_size=OUT1,
                    elem_step=OUT1)
                st = sb.tile([128, CHL, 128], f8, tag="sel")
                nc.sync.dma_start(
                    st[:].rearrange("p a b -> p (a b)"), slp_d[:, ts(t, EPL)])
                nc.vector.tensor_mul(
                    g[:], g[:],
                    wlp[:, ts(t, CHL)].unsqueeze(2).broadcast_to(
                        [128, CHL, OUT1]))
                pa = ps.tile([128, OUT1], f32, tag="acc")
                for ch in range(CHL):
                    nc.tensor.matmul(pa[:], lhsT=st[:, ch, :], rhs=g[:, ch, :],
                                     start=(ch == 0), stop=(ch == CHL - 1))
                res = sb.tile([128, OUT1], bf, tag="lres")
                nc.sync.dma_start(res[:], res_own[ts(t, 128), :])
                tf = sb.tile([128, OUT1], f32, tag="tf32")
                nc.vector.tensor_add(tf[:], pa[:], res[:])
                ob = sb.tile([128, OUT1], bf, tag="obuf")
                nc.vector.tensor_scalar(ob[:], tf[:], 0.5, 1.0,
                                        ALU.mult, ALU.min)
                nc.vector.tensor_scalar_max(ob[:], ob[:], 0.0)
                nc.sync.dma_start(out_own[ts(t, 128), :], ob[:])

        # ---- conv1: GAT + LP x2 ----
        gat_pass(h1p, h10_own, None)
        nc.gpsimd.collective_compute("AllGather", ALU.bypass,
                                     replica_groups=RG,
                                     ins=[h10_own[:]], outs=[h10[:]])
        lp_pass(h10, h10_own, h1a_own)
        nc.gpsimd.collective_compute("AllGather", ALU.bypass,
                                     replica_groups=RG,
                                     ins=[h1a_own[:]], outs=[h1a[:]])
        lp_pass(h1a, h10_own, h1f_own)

        # ---- GEMM2 (lhsT via DMA transpose of h1f_own) ----
        h1t = big.tile([128, KH, SLAB], bf, tag="glhs")
        for k in range(KH):
            nc.sync.dma_start(h1t[:, k, :], h1f_own[:, ts(k, 128)],
                              transpose=True)
        gemm(w2, KH, lambda k, t: h1t[:, k, ts(t, 128)], h2p_own)
        nc.gpsimd.collective_compute("AllGather", ALU.bypass,
                                     replica_groups=RG,
                                     ins=[h2p_own[:]], outs=[h2p[:]])

        # ---- conv2: GAT + LP x2 ----
        gat_pass(h2p, h20_own, None)
        nc.gpsimd.collective_compute("AllGather", ALU.bypass,
                                     replica_groups=RG,
                                     ins=[h20_own[:]], outs=[h20[:]])
        lp_pass(h20, h20_own, h2a_own)
        nc.gpsimd.collective_compute("AllGather", ALU.bypass,
                                     replica_groups=RG,
                                     ins=[h2a_own[:]], outs=[h2a[:]])
        lp_pass(h2a, h20_own, h2f_own)

        # ---- pooling (transposed): pooledT[f, g], one PSUM group per chunk --
        par = big.tile([128, FCH * NG], f32)
        psrc = ([(xr_d, k) for k in range(KIN)]
                + [(h1f_own, k) for k in range(KH)]
                + [(h2f_own, k) for k in range(KH)])
        for kk, (srcd, k) in enumerate(psrc):
            pk = pp.tile([128, NG], f32, tag="poolk")
            for t in range(TPC):
                lh = sb.tile([128, 128], bf, tag="plh")
                nc.sync.dma_start(lh[:], srcd[ts(t, 128), ts(k, 128)])
                nc.tensor.matmul(pk[:], lhsT=lh[:], rhs=spool[:, ts(t, NG)],
                                 start=(t == 0), stop=(t == TPC - 1))
            nc.vector.tensor_copy(par[:, ts(kk, NG)], pk[:])
        nc.sync.dma_start(ar_in[:], par[:])
        nc.gpsimd.collective_compute("AllReduce", ALU.add,
                                     replica_groups=RG,
                                     ins=[ar_in[:]], outs=[ar_out[:]])
        pool = big.tile([128, FCH, NG], f32)
        nc.sync.dma_start(pool[:].rearrange("p a b -> p (a b)"), ar_out[:])
        nc.vector.tensor_mul(
            pool[:], pool[:],
            rcnt[:].unsqueeze(1).broadcast_to([128, FCH, NG]))
        poolb = big.tile([128, FCH, NG], bf)
        nc.vector.tensor_copy(poolb[:], pool[:])

        # ---- MLP (transposed) ----
        mw1 = cst.tile([128, FCH, OC, 128], bf)
        nc.sync.dma_start(mw1[:].rearrange("p a b q -> p (a b q)"), mw1_d[:])
        mw2 = cst.tile([128, OC, 128], bf)
        nc.sync.dma_start(mw2[:].rearrange("p a b -> p (a b)"), mw2_d[:])
        hdd = big.tile([128, OC, NG], bf)
        for cc in range(OC):
            hps = pp.tile([128, NG], f32, tag="poolk")
            for k in range(FCH):
                nc.tensor.matmul(hps[:], lhsT=mw1[:, k, cc, :],
                                 rhs=poolb[:, k, :],
                                 start=(k == 0), stop=(k == FCH - 1))
            nc.vector.tensor_scalar(hdd[:, cc, :], hps[:],
                                    mb1[:, cc:cc + 1], 0.0, ALU.add, ALU.max)
        ops = pp.tile([128, NG], f32, tag="ops")
        for cc in range(OC):
            nc.tensor.matmul(ops[:], lhsT=mw2[:, cc, :], rhs=hdd[:, cc, :],
                             start=(cc == 0), stop=(cc == OC - 1))
        ofin = big.tile([128, NG], f32)
        nc.vector.tensor_scalar_add(ofin[:], ops[:], mb2[:, 0:1])
        nc.sync.dma_start(outT_d[:], ofin[:])

    nc.compile()
    return nc


# ---------------- host driver ----------------

def _prepare(x, edge_index, batch,
             conv1_W, conv1_asrc, conv1_adst, conv1_b,
             conv2_W, conv2_asrc, conv2_adst, conv2_b,
             mlp_W1, mlp_b1, mlp_W2, mlp_b2):
    c = CFG
    N, NPAD, IN_CH, OUT1, HEADS, HID = (c["N"], c["NPAD"], c["IN_CH"],
                                        c["OUT1"], c["HEADS"], c["HID"])
    NG, NCORE = c["N_GRAPHS"], c["NCORE"]
    NT = NPAD // 128
    TPC = NT // NCORE
    SLAB = TPC * 128
    pad_idx = N  # zero row

    src = np.asarray(edge_index[0], np.int64).astype(np.int32)
    dst = np.asarray(edge_index[1], np.int64).astype(np.int32)
    batch = np.asarray(batch, np.int64).astype(np.int32)
    loop = np.arange(N, dtype=np.int32)
    csrc = np.concatenate([src, loop])
    cdst = np.concatenate([dst, loop])

    deg = np.bincount(dst, minlength=N).astype(np.float32)
    dis = np.where(deg > 0, 1.0 / np.sqrt(np.maximum(deg, 1.0)), 0.0)
    wgt = dis[src] * dis[dst]

    EPG, gsp, gdp, gdl, _ = _edge_tiles(csrc, cdst, NT, pad_idx)
    EPL, lsp, _, ldl, lwp = _edge_tiles(src, dst, NT, pad_idx, wgt)
    Sg = _selector(gdl)
    Sl = _selector(ldl)

    # pooling one-hot
    bpad = np.full(NPAD, -1, np.int32)
    bpad[:N] = batch
    Spool = np.zeros((NT, 128, NG), np.uint8)
    ti = np.repeat(np.arange(NT), 128)
    pi = np.tile(np.arange(128), NT)
    v = bpad >= 0
    Spool[ti[v], pi[v], bpad[v]] = 1

    cnt = np.bincount(batch, minlength=NG).astype(np.float32)
    rcnt = (1.0 / np.maximum(cnt, 1.0)).astype(np.float32)

    x = np.asarray(x, np.float32)
    xp = np.zeros((NPAD, IN_CH), np.float32)
    xp[:N] = x
    xT = np.ascontiguousarray(xp.T).astype(BF16)
    xrows = xp.astype(BF16)

    def fold(W, asrc, adst):
        ws, wd = _fold_logit_w(np.asarray(W, np.float32),
                               np.asarray(asrc, np.float32),
                               np.asarray(adst, np.float32), HEADS, HID)
        return np.concatenate([np.asarray(W, np.float32), ws, wd],
                              axis=1).astype(BF16)

    w1e = fold(conv1_W, conv1_asrc, conv1_adst)
    w2e = fold(conv2_W, conv2_asrc, conv2_adst)
    b1r = np.tile(np.asarray(conv1_b, np.float32)[None, :],
                  (128, 1)).astype(BF16)
    b2r = np.tile(np.asarray(conv2_b, np.float32)[None, :],
                  (128, 1)).astype(BF16)

    W1 = np.asarray(mlp_W1, np.float32)     # [JK, 256]
    W2 = np.asarray(mlp_W2, np.float32)     # [256, 128]
    JK = W1.shape[0]
    FCH = JK // 128
    OC = W1.shape[1] // 128
    mw1 = np.ascontiguousarray(
        W1.reshape(FCH, 128, OC, 128).transpose(1, 0, 2, 3)
        .reshape(128, -1)).astype(BF16)
    mw2 = np.ascontiguousarray(
        W2.reshape(OC, 128, W2.shape[1]).transpose(1, 0, 2)
        .reshape(128, -1)).astype(BF16)
    mb1 = np.ascontiguousarray(
        np.asarray(mlp_b1, np.float32).reshape(OC, 128).T)
    mb2 = np.asarray(mlp_b2, np.float32).reshape(128, 1)
    rcT = np.tile(rcnt[None, :], (128, 1))

    in_maps = []
    for cix in range(NCORE):
        tl = slice(cix * TPC, (cix + 1) * TPC)
        rows = slice(cix * SLAB, (cix + 1) * SLAB)
        in_maps.append({
            "xts": np.ascontiguousarray(xT[:, rows]),
            "xr": np.ascontiguousarray(xrows[rows]),
            "w1e": w1e, "w2e": w2e, "b1r": b1r, "b2r": b2r,
            "gidx": _wrap_idx(gsp[tl]),
            "geidx": _wrap_idx(np.concatenate([gsp[tl], gdp[tl]], axis=1)),
            "sgat": _sel_layout(Sg[tl]),
            "lidx": _wrap_idx(lsp[tl]),
            "slp": _sel_layout(Sl[tl]),
            "wlp": _chunk_layout(lwp[tl]).astype(BF16),
            "spool": np.ascontiguousarray(
                Spool[tl].transpose(1, 0, 2).reshape(128, -1)).astype(F8),
            "mw1": mw1, "mw2": mw2, "mb1": mb1, "mb2": mb2, "rcnt": rcT,
        })
    return (EPG, EPL), in_maps


def kernel(x, edge_index, batch,
           conv1_W, conv1_asrc, conv1_adst, conv1_b,
           conv2_W, conv2_asrc, conv2_adst, conv2_b,
           mlp_W1, mlp_b1, mlp_W2, mlp_b2):
    NCORE = CFG["NCORE"]
    (EPG, EPL), in_maps = _prepare(
        x, edge_index, batch, conv1_W, conv1_asrc, conv1_adst, conv1_b,
        conv2_W, conv2_asrc, conv2_adst, conv2_b,
        mlp_W1, mlp_b1, mlp_W2, mlp_b2)
    key = (EPG, EPL, NCORE)
    if _cached.get("key") != key:
        _cached["nc"] = _build(key)
        _cached["key"] = key
    nc = _cached["nc"]

    if os.environ.get("BASS_KERNEL_SIM", "0") == "1":
        from concourse.bass_interp import MultiCoreSim
        nw = int(os.environ.get("BASS_SIM_WORKERS",
                                str(min(NCORE, os.cpu_count() or 1))))
        sim = MultiCoreSim(nc, num_cores=NCORE, require_finite=False,
                           num_workers=nw)
        for cix, cs in enumerate(sim.cores.values()):
            for k, v in in_maps[cix].items():
                cs.tensor(k)[:] = v
        sim.simulate()
        _cached["sim_time_ns"] = int(sim.global_time)
        outT = np.asarray(sim.cores[0].tensor("outT"))
    else:
        from concourse.bass_utils import run_bass_kernel_spmd
        import time
        t0 = time.time()
        res = run_bass_kernel_spmd(nc, in_maps, core_ids=list(range(NCORE)))
        _cached["device_wall_ns"] = int((time.time() - t0) * 1e9)
        _cached["last_result"] = res
        outT = res.results[0]["outT"]
    return np.ascontiguousarray(outT.T.astype(np.float32))
